# revision 1
# baseline (speedup 1.0000x reference)
"""Trainium2 Bass kernel for batched single-head attention with projections.

Reference computation (per batch b):
    Q = q @ Wq + bq ; K = k @ Wk + bk ; V = v @ Wv + bv        (512 -> 64)
    out = softmax(Q K^T / 8) V                                  (S = 4096)

Sharding: 8 cores = 4 batches x 2 query-sequence halves. Each core gets
its q half (transposed, bf16) plus the full k,v for its batch (transposed,
bf16, duplicated across the pair of cores that share the batch).

Device-side layout trick: everything is computed in "transposed space".
  Q.T [64, 2048]  = Wq.T @ qT   (+bq per-partition)
  K.T [64, 4096]  = Wk.T @ kT   (bk dropped: softmax-invariant)
  V'  [4096, 65]  = (vT.T @ Wv_aug) + bias ; col 64 == 1.0 (denominator col)
  scores.T tile   = K.T-chunk.T @ Q.T-block     -> PSUM [128, 512]
  P.T             = exp(scores.T / 8)           -> SBUF bf16 (ScalarE)
  out.T [65, 512] = sum_chunks V'-chunk.T @ P.T -> PSUM accumulate
Rows 0..63 of out.T are the unnormalized numerator, row 64 the softmax
denominator; the host divides and transposes while unsharding.

The scores matmul has contraction dim 64, so pairs of kv-tiles are packed
into the two 64-row halves of the PE array (tile_position row tiling) and
run concurrently; Q.T/K.T are duplicated into partitions 64..127 for this.
"""

import numpy as np
import ml_dtypes

import concourse.bass as bass
import concourse.tile as tile
from concourse import mybir
from concourse.bass_utils import run_bass_kernel_spmd

BF16 = mybir.dt.bfloat16
F32 = mybir.dt.float32

B, S, D, E = 4, 4096, 512, 64
H = S                 # q rows per core (full sequence)
KS = S // 2           # kv rows per core (half sequence)
E1 = E + 1            # V' width (ones column appended)
NCH = D // 128        # contraction chunks (4)
NKV = KS // 128       # kv tiles (16)
NPAIR = NKV // 2      # packed kv tile pairs (8)
QBLK = 512            # sq columns per block
NBLK = H // QBLK      # 8
N_CORES = 8


def _build_bass(split_waits: bool = True) -> bass.Bass:
    nc = bass.Bass()
    qT = nc.declare_dram_parameter("qT", [D, H], BF16, isOutput=False)
    kT = nc.declare_dram_parameter("kT", [D, KS], BF16, isOutput=False)
    vT = nc.declare_dram_parameter("vT", [D, KS], BF16, isOutput=False)
    # weights pre-swizzled on host to [128, chunk*width] (partition-major)
    wq = nc.declare_dram_parameter("wq", [128, NCH * E], BF16, isOutput=False)
    wk = nc.declare_dram_parameter("wk", [128, NCH * E], BF16, isOutput=False)
    wv = nc.declare_dram_parameter("wv", [128, NCH * E1], BF16, isOutput=False)
    bq = nc.declare_dram_parameter("bqb", [128, 512], BF16, isOutput=False)
    bvb = nc.declare_dram_parameter("bvb", [128, E1], F32, isOutput=False)
    out = nc.declare_dram_parameter("out", [E1, H], F32, isOutput=True)

    with tile.TileContext(nc) as tc:
        _body(nc, tc, qT, kT, vT, wq, wk, wv, bq, bvb, out)
    if split_waits:
        _split_multi_waits(nc)
    return nc


_NO_SPLIT_OPCODES = {"Drain", "EventSemaphore", "NoOp", "Call", "ISA",
                     "UnconditionalBranch"}


def _split_multi_waits(nc):
    """walrus (this toolchain) encodes at most ONE sem wait per TPB
    instruction (single NEURON_ISA_TPB_EVENTS slot) and refuses to compile
    instructions carrying more. Tile emits multi-wait sync_info freely, so
    split: keep the first wait on the instruction, hoist the rest onto
    standalone EventSemaphore waits just before it on the same engine."""
    n = 0
    for blk in nc.m.functions[0].blocks:
        new_insts = []
        for inst in blk.instructions:
            si = inst.sync_info
            if (si is not None and si.on_wait and len(si.on_wait) > 1
                    and inst.concise_opcode not in _NO_SPLIT_OPCODES):
                waits = list(si.on_wait)
                for w in waits[:-1]:
                    n += 1
                    es = mybir.InstEventSemaphore(
                        name=f"WSPLIT-{n}", ins=[], outs=[])
                    es.engine = inst.engine
                    es.sync_info = mybir.SyncInfo(on_wait=[w], on_update=[])
                    new_insts.append(es)
                inst.sync_info = mybir.SyncInfo(
                    on_wait=[waits[-1]], on_update=list(si.on_update))
            new_insts.append(inst)
        blk.instructions = new_insts


def _body(nc, tc, qT, kT, vT, wq, wk, wv, bq, bvb, out):
    with (
        tc.tile_pool(name="consts", bufs=1) as cst,
        tc.tile_pool(name="raw", bufs=1) as raw,
        tc.tile_pool(name="proj", bufs=1) as proj,
        tc.tile_pool(name="pt", bufs=8) as ptp,
        tc.tile_pool(name="ob", bufs=2) as obp,
        tc.tile_pool(name="ps", bufs=2, space="PSUM") as ps,
        tc.tile_pool(name="pso", bufs=2, space="PSUM") as pso,
    ):
        # consts ride the gpsimd SWDGE ring so they never add fixed
        # per-DMA latency ahead of the big input loads on the HWDGE rings
        wk_sb = cst.tile([128, NCH * E], BF16, tag="wk")
        nc.gpsimd.dma_start(out=wk_sb, in_=wk[:, :])

        # raw transposed inputs. The HWDGE ring serializes ~1.5us of fixed
        # cost per dma_start, so loads are consolidated: one 3D-AP DMA
        # fills all 4 contraction-chunk slices at once. Two rings: sync
        # carries kT+qT (the critical chain to the first scores matmul),
        # scalar carries vT. First kv quarter is a separate DMA so the
        # attention pipeline starts before the bulk lands.
        QW = 1024

        def load3d(eng, name, src, c0, c1, w):
            t = raw.tile([128, NCH, w], BF16, tag=name)
            eng.dma_start(
                out=t,
                in_=src[:, c0:c1].rearrange("(c p) w -> p c w", p=128))
            return t

        wq_sb = cst.tile([128, NCH * E], BF16, tag="wq")
        nc.gpsimd.dma_start(out=wq_sb, in_=wq[:, :])
        bq_sb = cst.tile([128, 512], BF16, tag="bq")
        nc.gpsimd.dma_start(out=bq_sb, in_=bq[:, :])
        qt_b0 = load3d(nc.sync, "qt_b0", qT, 0, 512, 512)
        kq0a = load3d(nc.sync, "kq0a", kT, 0, 512, 512)
        kq0b = load3d(nc.sync, "kq0b", kT, 512, QW, 512)
        kq1 = load3d(nc.sync, "kq1", kT, QW, 2 * QW, QW)
        qt_r1 = load3d(nc.sync, "qt_r1", qT, 512, 2048, 1536)
        qt_r2 = load3d(nc.sync, "qt_r2", qT, 2048, H, H - 2048)

        wv_sb = cst.tile([128, NCH * E1], BF16, tag="wv")
        nc.gpsimd.dma_start(out=wv_sb, in_=wv[:, :])
        bvb_sb = cst.tile([128, E1], F32, tag="bvb")
        nc.gpsimd.dma_start(out=bvb_sb, in_=bvb[:, :])
        # preload the exp table set off the critical path (first real exp
        # otherwise eats the ~2.7us ACT_TABLE_LOAD mid-pipeline)
        scr = cst.tile([1, 8], F32, tag="scr")
        nc.scalar.activation(scr[:, :], bvb_sb[0:1, 0:8],
                             mybir.ActivationFunctionType.Exp)
        vq = [None] * 2
        for q in range(2):
            vq[q] = load3d(nc.scalar, f"vq{q}", vT, q * QW, (q + 1) * QW, QW)

        def qt_slice(c, blk):
            if blk == 0:
                return qt_b0[:, c, :]
            if blk < 4:
                return qt_r1[:, c, (blk - 1) * 512:blk * 512]
            return qt_r2[:, c, (blk - 4) * 512:(blk - 3) * 512]

        def kt_slice(c, blk):       # 512-col K projection block
            if blk == 0:
                return kq0a[:, c, :]
            if blk == 1:
                return kq0b[:, c, :]
            return kq1[:, c, (blk - 2) * 512:(blk - 1) * 512]

        def vt_slice(c, t):         # 128-col V tile
            return vq[t // 8][:, c, (t % 8) * 128:(t % 8 + 1) * 128]


        # projected tensors; Q.T/K.T duplicated into partitions 64..127 so
        # the scores matmuls can row-pack both PE array halves
        QT2 = proj.tile([128, H], BF16, tag="QT2")
        KT2 = proj.tile([128, KS], BF16, tag="KT2")
        Vp = proj.tile([128, NKV, E1], BF16, tag="Vp")

        # projections, emitted in dependency-arrival order; the dup DMAs
        # (partitions 64..127 copies) ride the ACT DGE ring so they never
        # queue behind the big input loads on the sync ring
        def q_proj(blk):
            acc = ps.tile([128, 512], F32, tag="ps_main")
            sl = slice(blk * 512, (blk + 1) * 512)
            for c in range(NCH):
                nc.tensor.matmul(
                    acc[0:E, :], wq_sb[:, c * E:(c + 1) * E], qt_slice(c, blk),
                    start=(c == 0), stop=(c == NCH - 1),
                    tile_position=(0, 0),
                )
            for c in range(NCH):
                nc.tensor.matmul(
                    acc[E:2 * E, :], wq_sb[:, c * E:(c + 1) * E],
                    qt_slice(c, blk),
                    start=(c == 0), stop=(c == NCH - 1),
                    tile_position=(0, 64),
                )
            nc.vector.tensor_add(QT2[:, sl], acc[:, :], bq_sb[:, :])

        def k_proj(blk):
            acc = ps.tile([128, 512], F32, tag="ps_main")
            sl = slice(blk * 512, (blk + 1) * 512)
            for c in range(NCH):
                nc.tensor.matmul(
                    acc[0:E, :], wk_sb[:, c * E:(c + 1) * E], kt_slice(c, blk),
                    start=(c == 0), stop=(c == NCH - 1),
                    tile_position=(0, 0),
                )
            for c in range(NCH):
                nc.tensor.matmul(
                    acc[E:2 * E, :], wk_sb[:, c * E:(c + 1) * E],
                    kt_slice(c, blk),
                    start=(c == 0), stop=(c == NCH - 1),
                    tile_position=(0, 64),
                )
            nc.vector.tensor_copy(KT2[:, sl], acc[:, :])

        def v_proj(t):
            acc = ps.tile([128, E1], F32, tag="ps_main")
            for c in range(NCH):
                nc.tensor.matmul(
                    acc[:, :], vt_slice(c, t),
                    wv_sb[:, c * E1:(c + 1) * E1],
                    start=(c == 0), stop=(c == NCH - 1),
                )
            nc.vector.tensor_add(Vp[:, t, :], acc[:, :], bvb_sb[:, :])

        # attention pair: scores (row-packed kv-tile pair) -> exp -> AV
        def pair(blk, p, acc_o):
            sq = slice(blk * QBLK, (blk + 1) * QBLK)
            sc = ps.tile([128, 2 * QBLK], F32, tag="ps_sc")
            nc.tensor.matmul(
                sc[:, 0:QBLK],
                KT2[0:E, (2 * p) * 128:(2 * p + 1) * 128],
                QT2[0:E, sq],
                start=True, stop=True, tile_position=(0, 0),
            )
            nc.tensor.matmul(
                sc[:, QBLK:2 * QBLK],
                KT2[E:2 * E, (2 * p + 1) * 128:(2 * p + 2) * 128],
                QT2[E:2 * E, sq],
                start=True, stop=True, tile_position=(64, 0),
            )
            pt = ptp.tile([128, 2 * QBLK], BF16, tag="pt")
            nc.scalar.activation(
                pt[:, :], sc[:, :], mybir.ActivationFunctionType.Exp,
                scale=0.125,
            )
            nc.tensor.matmul(
                acc_o[:, :], Vp[:, 2 * p, :], pt[:, 0:QBLK],
                start=(p == 0), stop=False,
            )
            nc.tensor.matmul(
                acc_o[:, :], Vp[:, 2 * p + 1, :], pt[:, QBLK:2 * QBLK],
                start=False, stop=(p == NPAIR - 1),
            )

        def epilogue(blk, acc_o):
            sq = slice(blk * QBLK, (blk + 1) * QBLK)
            ob = obp.tile([E1, QBLK], F32, tag="ob")
            nc.vector.tensor_copy(ob[:, :], acc_o[:, :])
            nc.scalar.dma_start(out=out[:, sq], in_=ob[:, :])

        # block 0 interleaves projection waves with attention pairs so the
        # PE's in-order stream matches DMA arrival (kv quarter w feeds
        # pairs 4w..4w+3); blocks 1-3 are pure attention reusing K.T/V'.
        k_proj(0); k_proj(1); q_proj(0)
        v_proj(0); v_proj(1)
        acc_o = pso.tile([E1, QBLK], F32, tag="ps_out")
        pair(0, 0, acc_o)
        v_proj(2); v_proj(3)
        pair(0, 1, acc_o)
        v_proj(4); v_proj(5)
        pair(0, 2, acc_o)
        v_proj(6); v_proj(7)
        pair(0, 3, acc_o)
        k_proj(2); k_proj(3)
        for t in range(8, 16):
            v_proj(t)
        for p in range(4, NPAIR):
            pair(0, p, acc_o)
            if p == 4:
                q_proj(1)
        epilogue(0, acc_o)
        for blk in range(1, NBLK):
            acc_o = pso.tile([E1, QBLK], F32, tag="ps_out")
            if blk + 1 < NBLK:
                q_proj(blk + 1)
            for p in range(NPAIR):
                pair(blk, p, acc_o)
            epilogue(blk, acc_o)


_CACHED_NC = None


def _get_nc():
    global _CACHED_NC
    if _CACHED_NC is None:
        _CACHED_NC = _build_bass()
    return _CACHED_NC


def _swizzle_w(w: np.ndarray) -> np.ndarray:
    """[512, width] -> [128, NCH*width] with chunk-major free dim."""
    width = w.shape[1]
    return np.ascontiguousarray(
        w.reshape(NCH, 128, width).transpose(1, 0, 2).reshape(128, NCH * width)
    ).astype(ml_dtypes.bfloat16)


def _make_in_maps(q, k, v, Wq, bq, Wk, bk, Wv, bv):
    del bk  # constant along the kv axis -> softmax-invariant, dropped
    bf = ml_dtypes.bfloat16
    wq_s = _swizzle_w(np.asarray(Wq, np.float32))
    wk_s = _swizzle_w(np.asarray(Wk, np.float32))
    wv_aug = np.concatenate(
        [np.asarray(Wv, np.float32), np.zeros((D, 1), np.float32)], axis=1
    )
    wv_s = _swizzle_w(wv_aug)
    bq_col = np.asarray(bq, np.float32).reshape(E, 1)
    bq_a = np.ascontiguousarray(np.broadcast_to(
        np.concatenate([bq_col, bq_col], axis=0), (2 * E, 512))).astype(bf)
    bvb_row = np.concatenate([np.asarray(bv, np.float32), [1.0]]).astype(np.float32)
    bvb_a = np.ascontiguousarray(np.broadcast_to(bvb_row, (128, E1)))

    in_maps = []
    for core in range(N_CORES):
        b, h = core // 2, core % 2
        kh = np.asarray(k[b, h * KS:(h + 1) * KS, :], np.float32)
        vh = np.asarray(v[b, h * KS:(h + 1) * KS, :], np.float32)
        in_maps.append({
            "qT": np.ascontiguousarray(np.asarray(q[b], np.float32).T).astype(bf),
            "kT": np.ascontiguousarray(kh.T).astype(bf),
            "vT": np.ascontiguousarray(vh.T).astype(bf),
            "wq": wq_s, "wk": wk_s, "wv": wv_s,
            "bqb": bq_a, "bvb": bvb_a,
        })
    return in_maps


def _unshard(results) -> np.ndarray:
    final = np.empty((B, S, E), np.float32)
    for b in range(B):
        o = (np.asarray(results[2 * b]["out"], np.float32)
             + np.asarray(results[2 * b + 1]["out"], np.float32))  # [65, S]
        final[b] = (o[:E] / o[E:E + 1]).T
    return final


def kernel(q, k, v, Wq, bq, Wk, bk, Wv, bv, _trace=False):
    nc = _get_nc()
    in_maps = _make_in_maps(q, k, v, Wq, bq, Wk, bk, Wv, bv)
    res = run_bass_kernel_spmd(nc, in_maps, core_ids=list(range(N_CORES)),
                               trace=_trace)
    outp = _unshard(res.results)
    if _trace:
        kernel.last_result = res
    return outp



# revision 8
# speedup vs baseline: 1.0964x; 1.0964x over previous
"""Trainium2 Bass kernel for batched single-head attention with projections.

Reference computation (per batch b):
    Q = q @ Wq + bq ; K = k @ Wk + bk ; V = v @ Wv + bv        (512 -> 64)
    out = softmax(Q K^T / 8) V                                  (S = 4096)

Sharding: 8 cores = 4 batches x 2 kv-sequence halves. Each core gets
its full q (transposed, bf16) plus half of k,v for its batch (transposed,
bf16). Cores emit unnormalized numerator + denominator; host combines.

Device-side layout (transposed space):
  Q.T [128, 4096] = (Wq|Wq).T @ qT (+bq)   rows 64..127 duplicate 0..63
  K.T [128, 2048] = (Wk|Wk).T @ kT         (bk dropped: softmax-invariant)
  V'  [2048, 65]  = vT.T @ Wv_aug + bias ; col 64 == 1.0 (denominator col)
  per kv-tile T (128 kv rows x 512 q): scores.T -> PSUM, exp -> bf16 SBUF,
  V'.T @ P.T accumulated into [65, 512] per q-block.

Perf structure (v2):
  - scores PSUM = two 3-bank slots [128, 1536]; one ACTIVATE(exp) per
    3-tile group (N=1536) -> ScalarE runs near its floor.
  - kv-tiles pair by parity into PE row halves (tile_position row tiling)
    so the two 64-contraction scores MMs run concurrently.
  - projections use column-duplicated weights (one 128-wide stationary)
    -> half the matmul/LDW count of the col-tiled variant.
  - input DMAs fan out across sync/scalar/vector rings at t=0; weights +
    V + output stores ride the gpsimd SWDGE ring; ScalarE does exps only.
"""

import numpy as np
import ml_dtypes

import concourse.bass as bass
import concourse.tile as tile
from concourse import mybir
from concourse.bass_utils import run_bass_kernel_spmd

BF16 = mybir.dt.bfloat16
F32 = mybir.dt.float32

B, S, D, E = 4, 4096, 512, 64
H = S                 # q rows per core (full sequence)
KS = S // 2           # kv rows per core (half sequence)
E1 = E + 1            # V' width (ones column appended)
NCH = D // 128        # contraction chunks (4)
NKV = KS // 128       # kv tiles per core (16)
QBLK = 512            # q columns per block
NBLK = H // QBLK      # 8
NT = NBLK * NKV       # global tile count (128)
GRP = 3               # kv-tiles per exp group / psum slot
N_CORES = 8


def _build_bass(split_waits: bool = True) -> bass.Bass:
    nc = bass.Bass()
    qT = nc.declare_dram_parameter("qT", [D, H], BF16, isOutput=False)
    kT = nc.declare_dram_parameter("kT", [D, KS], BF16, isOutput=False)
    vT = nc.declare_dram_parameter("vT", [D, KS], BF16, isOutput=False)
    # weights pre-swizzled on host to [128, chunk*width] (partition-major);
    # wq/wk have their 64 columns duplicated -> 128-wide stationary
    wq = nc.declare_dram_parameter("wq", [128, NCH * 2 * E], BF16, isOutput=False)
    wk = nc.declare_dram_parameter("wk", [128, NCH * 2 * E], BF16, isOutput=False)
    wv = nc.declare_dram_parameter("wv", [128, NCH * E1], BF16, isOutput=False)
    bq = nc.declare_dram_parameter("bqb", [128, QBLK], BF16, isOutput=False)
    bvb = nc.declare_dram_parameter("bvb", [128, E1], F32, isOutput=False)
    out = nc.declare_dram_parameter("out", [E1, H], F32, isOutput=True)

    with tile.TileContext(nc) as tc:
        _body(nc, tc, qT, kT, vT, wq, wk, wv, bq, bvb, out)
    if split_waits:
        _split_multi_waits(nc)
    return nc


_NO_SPLIT_OPCODES = {"Drain", "EventSemaphore", "NoOp", "Call", "ISA",
                     "UnconditionalBranch"}


def _split_multi_waits(nc):
    """walrus (this toolchain) encodes at most ONE sem wait per TPB
    instruction (single NEURON_ISA_TPB_EVENTS slot) and refuses to compile
    instructions carrying more. Tile emits multi-wait sync_info freely, so
    split: keep the first wait on the instruction, hoist the rest onto
    standalone EventSemaphore waits just before it on the same engine."""
    n = 0
    for blk in nc.m.functions[0].blocks:
        new_insts = []
        for inst in blk.instructions:
            si = inst.sync_info
            if (si is not None and si.on_wait and len(si.on_wait) > 1
                    and inst.concise_opcode not in _NO_SPLIT_OPCODES):
                waits = list(si.on_wait)
                for w in waits[:-1]:
                    n += 1
                    es = mybir.InstEventSemaphore(
                        name=f"WSPLIT-{n}", ins=[], outs=[])
                    es.engine = inst.engine
                    es.sync_info = mybir.SyncInfo(on_wait=[w], on_update=[])
                    new_insts.append(es)
                inst.sync_info = mybir.SyncInfo(
                    on_wait=[waits[-1]], on_update=list(si.on_update))
            new_insts.append(inst)
        blk.instructions = new_insts
    return nc


def _body(nc, tc, qT, kT, vT, wq, wk, wv, bq, bvb, out):
    with (
        tc.tile_pool(name="consts", bufs=1) as cst,
        tc.tile_pool(name="raw", bufs=1) as raw,
        tc.tile_pool(name="proj", bufs=1) as proj,
        tc.tile_pool(name="pt", bufs=4) as ptp,
        tc.tile_pool(name="ob", bufs=2) as obp,
        tc.tile_pool(name="sc", bufs=2, space="PSUM") as scp,
        tc.tile_pool(name="acc", bufs=1, space="PSUM") as accp,
        tc.tile_pool(name="pp", bufs=1, space="PSUM") as ppp,
    ):
        # --- input DMAs, fanned out across rings so descriptor generation
        # (~1.9us per dma_start per ring) happens in parallel.
        def load3d(eng, name, src, c0, c1):
            w = c1 - c0
            t = raw.tile([128, NCH, w], BF16, tag=name)
            eng.dma_start(
                out=t,
                in_=src[:, c0:c1].rearrange("(c p) w -> p c w", p=128))
            return t

        # gpsimd (SWDGE): small weight tensors first
        wk_sb = cst.tile([128, NCH * 2 * E], BF16, tag="wk")
        nc.gpsimd.dma_start(out=wk_sb, in_=wk[:, :])
        wq_sb = cst.tile([128, NCH * 2 * E], BF16, tag="wq")
        nc.gpsimd.dma_start(out=wq_sb, in_=wq[:, :])
        bq_sb = cst.tile([128, QBLK], BF16, tag="bq")
        nc.gpsimd.dma_start(out=bq_sb, in_=bq[:, :])

        # sync ring: K first (feeds the first scores), then the q tail
        kq = [load3d(nc.sync, f"kq{i}", kT, i * 512, (i + 1) * 512)
              for i in range(4)]
        qt_r1 = load3d(nc.sync, "qt_r1", qT, 512, 2048)
        qt_r2 = load3d(nc.sync, "qt_r2", qT, 2048, H)

        # scalar ring: q block 0 + V first half, then exps only
        qt_b0 = load3d(nc.scalar, "qt_b0", qT, 0, 512)
        vq0 = load3d(nc.scalar, "vq0", vT, 0, 1024)

        # gpsimd: rest of the consts + V second half
        wv_sb = cst.tile([128, NCH * E1], BF16, tag="wv")
        nc.gpsimd.dma_start(out=wv_sb, in_=wv[:, :])
        bvb_sb = cst.tile([128, E1], F32, tag="bvb")
        nc.gpsimd.dma_start(out=bvb_sb, in_=bvb[:, :])
        vq1 = load3d(nc.gpsimd, "vq1", vT, 1024, 2048)

        # preload the exp table set off the critical path (first real exp
        # otherwise eats the ~2.7us ACT_TABLE_LOAD mid-pipeline)
        scr = cst.tile([1, 8], F32, tag="scr")
        nc.scalar.activation(scr[:, :], wk_sb[0:1, 0:8],
                             mybir.ActivationFunctionType.Exp)

        def qt_slice(c, blk):
            if blk == 0:
                return qt_b0[:, c, :]
            if blk < 4:
                return qt_r1[:, c, (blk - 1) * 512:blk * 512]
            return qt_r2[:, c, (blk - 4) * 512:(blk - 3) * 512]

        def vt_slice(c, t):         # 128-col V tile
            return (vq0 if t < 8 else vq1)[:, c, (t % 8) * 128:(t % 8 + 1) * 128]

        # projected tensors; Q.T/K.T rows 64..127 duplicate rows 0..63 (via
        # column-duplicated weights) so scores matmuls can row-pack.
        QT2 = proj.tile([128, H], BF16, tag="QT2")
        KT2 = proj.tile([128, KS], BF16, tag="KT2")
        Vp = proj.tile([128, NKV, E1], BF16, tag="Vp")

        def q_proj(blk):
            acc = ppp.tile([128, QBLK], F32, tag="pp")
            sl = slice(blk * QBLK, (blk + 1) * QBLK)
            for c in range(NCH):
                nc.tensor.matmul(
                    acc[:, :], wq_sb[:, c * 128:(c + 1) * 128],
                    qt_slice(c, blk),
                    start=(c == 0), stop=(c == NCH - 1))
            nc.vector.tensor_add(QT2[:, sl], acc[:, :], bq_sb[:, :])

        def k_proj(blk):
            acc = ppp.tile([128, QBLK], F32, tag="pp")
            sl = slice(blk * QBLK, (blk + 1) * QBLK)
            for c in range(NCH):
                nc.tensor.matmul(
                    acc[:, :], wk_sb[:, c * 128:(c + 1) * 128],
                    kq[blk][:, c, :],
                    start=(c == 0), stop=(c == NCH - 1))
            nc.vector.tensor_copy(KT2[:, sl], acc[:, :])

        def v_proj(t):
            acc = ppp.tile([128, QBLK], F32, tag="pp")
            for c in range(NCH):
                nc.tensor.matmul(
                    acc[:, 0:E1], vt_slice(c, t),
                    wv_sb[:, c * E1:(c + 1) * E1],
                    start=(c == 0), stop=(c == NCH - 1))
            nc.vector.tensor_add(Vp[:, t, :], acc[:, 0:E1], bvb_sb[:, :])

        # --- attention, tiled over global kv-tile index T = blk*NKV + t.
        # Groups of GRP tiles share one psum slot + one exp ACTIVATE.
        state = {"sc": None, "acc": None}
        pt_of = {}

        def scores(T):
            blk, t = divmod(T, NKV)
            p = T % GRP
            if p == 0:
                state["sc"] = scp.tile([128, GRP * QBLK], F32, tag="sc", name="sc")
            sq = slice(blk * QBLK, (blk + 1) * QBLK)
            half = T % 2
            nc.tensor.matmul(
                state["sc"][:, p * QBLK:(p + 1) * QBLK],
                KT2[half * E:(half + 1) * E, t * 128:(t + 1) * 128],
                QT2[half * E:(half + 1) * E, sq],
                start=True, stop=True, tile_position=(half * E, 0))

        def exp_group(T):           # T = last tile of the group
            n = (T % GRP) + 1
            pt = ptp.tile([128, GRP * QBLK], BF16, tag="pt", name="pt")
            nc.scalar.activation(
                pt[:, 0:n * QBLK], state["sc"][:, 0:n * QBLK],
                mybir.ActivationFunctionType.Exp, scale=0.125)
            pt_of[T // GRP] = pt

        def av(T):
            blk, t = divmod(T, NKV)
            p = T % GRP
            if t == 0:
                state["acc"] = accp.tile([E1, QBLK], F32, tag="acc", name="acc")
            nc.tensor.matmul(
                state["acc"][:, :], Vp[:, t, :],
                pt_of[T // GRP][:, p * QBLK:(p + 1) * QBLK],
                start=(t == 0), stop=(t == NKV - 1))

        def epilogue(blk):
            sq = slice(blk * QBLK, (blk + 1) * QBLK)
            ob = obp.tile([E1, QBLK], F32, tag="ob")
            nc.vector.tensor_copy(ob[:, :], state["acc"][:, :])
            nc.gpsimd.dma_start(out=out[:, sq], in_=ob[:, :])

        # --- schedule. Groups of GRP tiles: scores(g) -> exp(g) -> av(g)
        # emitted with av lagging one group so the PE never queue-stalls
        # behind an ACTIVATE it doesn't depend on. Prologue interleaves
        # the projections in DMA-arrival order.
        def sc_group(g):
            for T in range(g * GRP, min((g + 1) * GRP, NT)):
                scores(T)
            exp_group(min((g + 1) * GRP, NT) - 1)

        def av_group(g):
            for T in range(g * GRP, min((g + 1) * GRP, NT)):
                ab, at = divmod(T, NKV)
                if at == 0 and ab > 0:
                    epilogue(ab - 1)
                    if ab + 1 < NBLK:
                        q_proj(ab + 1)
                av(T)

        k_proj(0)
        q_proj(0)
        sc_group(0)
        v_proj(0); v_proj(1); v_proj(2)
        k_proj(1)
        sc_group(1)
        av_group(0)
        v_proj(3); v_proj(4); v_proj(5)
        k_proj(2)
        sc_group(2)
        av_group(1)
        v_proj(6); v_proj(7); v_proj(8)
        k_proj(3)
        sc_group(3)
        av_group(2)
        v_proj(9); v_proj(10); v_proj(11)
        q_proj(1)
        sc_group(4)
        av_group(3)
        v_proj(12); v_proj(13); v_proj(14); v_proj(15)

        NGRP = (NT + GRP - 1) // GRP    # 43
        for g in range(5, NGRP):
            sc_group(g)
            av_group(g - 1)
        av_group(NGRP - 1)
        epilogue(NBLK - 1)


_CACHED_NC = None


def _get_nc():
    global _CACHED_NC
    if _CACHED_NC is None:
        _CACHED_NC = _build_bass()
    return _CACHED_NC


def _swizzle_w(w: np.ndarray) -> np.ndarray:
    """[512, width] -> [128, NCH*width] with chunk-major free dim."""
    width = w.shape[1]
    return np.ascontiguousarray(
        w.reshape(NCH, 128, width).transpose(1, 0, 2).reshape(128, NCH * width)
    ).astype(ml_dtypes.bfloat16)


def _make_in_maps(q, k, v, Wq, bq, Wk, bk, Wv, bv):
    del bk  # constant along the kv axis -> softmax-invariant, dropped
    bf = ml_dtypes.bfloat16
    wq_d = np.concatenate([np.asarray(Wq, np.float32)] * 2, axis=1)
    wk_d = np.concatenate([np.asarray(Wk, np.float32)] * 2, axis=1)
    wq_s = _swizzle_w(wq_d)
    wk_s = _swizzle_w(wk_d)
    wv_aug = np.concatenate(
        [np.asarray(Wv, np.float32), np.zeros((D, 1), np.float32)], axis=1
    )
    wv_s = _swizzle_w(wv_aug)
    bq_col = np.asarray(bq, np.float32).reshape(E, 1)
    bq_a = np.ascontiguousarray(np.broadcast_to(
        np.concatenate([bq_col, bq_col], axis=0), (2 * E, QBLK))).astype(bf)
    bvb_row = np.concatenate([np.asarray(bv, np.float32), [1.0]]).astype(np.float32)
    bvb_a = np.ascontiguousarray(np.broadcast_to(bvb_row, (128, E1)))

    in_maps = []
    for core in range(N_CORES):
        b, h = core // 2, core % 2
        kh = np.asarray(k[b, h * KS:(h + 1) * KS, :], np.float32)
        vh = np.asarray(v[b, h * KS:(h + 1) * KS, :], np.float32)
        in_maps.append({
            "qT": np.ascontiguousarray(np.asarray(q[b], np.float32).T).astype(bf),
            "kT": np.ascontiguousarray(kh.T).astype(bf),
            "vT": np.ascontiguousarray(vh.T).astype(bf),
            "wq": wq_s, "wk": wk_s, "wv": wv_s,
            "bqb": bq_a, "bvb": bvb_a,
        })
    return in_maps


def _unshard(results) -> np.ndarray:
    final = np.empty((B, S, E), np.float32)
    for b in range(B):
        o = (np.asarray(results[2 * b]["out"], np.float32)
             + np.asarray(results[2 * b + 1]["out"], np.float32))  # [65, S]
        final[b] = (o[:E] / o[E:E + 1]).T
    return final


def kernel(q, k, v, Wq, bq, Wk, bk, Wv, bv, _trace=False):
    nc = _get_nc()
    in_maps = _make_in_maps(q, k, v, Wq, bq, Wk, bk, Wv, bv)
    res = run_bass_kernel_spmd(nc, in_maps, core_ids=list(range(N_CORES)),
                               trace=_trace)
    outp = _unshard(res.results)
    if _trace:
        kernel.last_result = res
    return outp
